# revision 6
# baseline (speedup 1.0000x reference)
"""Trainium2 Bass kernel for nn_Attention_43181601194684.

Reference computation:
    h_last  = hidden[0, 1]                          # [B, H]
    proj    = einsum('blh,oh->blo', enc, W) + b     # [B, L, H]
    energies= einsum('bh,blh->bl', h_last, proj)    # [B, L]
    out     = softmax(energies, axis=1)[:, None, :] # [B, 1, L]

Algebraic simplification:
    energies[b, l] = (h_last[b] @ W) . enc[b, l] + (h_last[b] . bias)
The per-batch constant cancels inside the softmax, so the device kernel
computes   e[b, l] = v[b] . enc[b, l]   with v = h_last @ W, followed by a
numerically-stable softmax over l.

v is a [B, H] = 64 KiB tensor produced from the tiny [B,H]x[H,H] GEMM; it is
computed on the host and shipped pre-broadcast ([128, B_LOC, H], 1 MiB/core)
so the device spends zero instructions and zero critical-path latency on it.
The device is purely the memory-bound part: stream the 32 MiB/core encoder
slice, fused multiply+row-reduce on the DVE, per-batch softmax.

Sharding: data-parallel over batch. 32 batches / 8 cores = 4 per core.

Layout choices (all DMAs are long contiguous runs):
  - enc chunk c of batch b covers l in [c*1024, (c+1)*1024): partition p
    holds the 8 consecutive rows l = c*1024 + p*8 + k, i.e. a 16 KiB
    contiguous DRAM run per partition and a fully contiguous 2 MiB chunk.
  - the [128, 32] per-batch probability tile is stored as-is (contiguous
    16 KiB); the host inverts the (c,p,k) permutation with a numpy reshape.
"""

import numpy as np

B, L, H = 32, 4096, 512
N_CORES = 8
B_LOC = B // N_CORES  # 4
P = 128               # SBUF partitions
JCH = 8               # l-rows per partition per DMA chunk (2 MiB per DMA)
NCH = L // (P * JCH)  # 4 chunks per batch
NCOL = L // P         # 32 energy columns per batch

_PROGRAM = None


def _build_program():
    """Build + compile the single-core Bass/Tile program (SPMD across 8 cores)."""
    from contextlib import ExitStack

    import concourse.bacc as bacc
    import concourse.mybir as mybir
    import concourse.tile as tile
    from concourse.masks import make_identity

    fp32 = mybir.dt.float32
    Alu = mybir.AluOpType
    Act = mybir.ActivationFunctionType

    nc = bacc.Bacc("TRN2", target_bir_lowering=False, debug=False,
                   num_devices=N_CORES)

    enc = nc.dram_tensor("enc", [B_LOC, L, H], fp32, kind="ExternalInput")
    vr = nc.dram_tensor("vr", [B_LOC, P, H], fp32, kind="ExternalInput")
    probs = nc.dram_tensor("probs", [B_LOC, P, NCOL], fp32,
                           kind="ExternalOutput")

    POOL_KS = ()      # k-slices computed on the Pool engine, rest on DVE

    with tile.TileContext(nc) as tc, ExitStack() as ctx:
        consts = ctx.enter_context(tc.tile_pool(name="consts", bufs=1))
        wpool = ctx.enter_context(tc.tile_pool(name="wpool", bufs=1))
        epool = ctx.enter_context(tc.tile_pool(name="epool", bufs=8))
        scratch = ctx.enter_context(tc.tile_pool(name="scratch", bufs=2))
        pscratch = ctx.enter_context(tc.tile_pool(name="pscratch", bufs=2))
        epers = ctx.enter_context(tc.tile_pool(name="epers", bufs=1))
        small = ctx.enter_context(tc.tile_pool(name="small", bufs=2))
        psum = ctx.enter_context(tc.tile_pool(name="psum", bufs=2, space="PSUM"))

        # v for batch 0 lands first (256 KiB flat), then the first chunk's
        # sub-slices, so the first STT can fire as early as possible.
        v_sb = {}
        with tc.high_priority():
            v_sb[0] = wpool.tile([P, H], fp32, tag="v0", name="v0")
            nc.sync.dma_start(v_sb[0][:], vr[0])
            identity = consts.tile([P, P], fp32, tag="identity")
            make_identity(nc, identity)
            ones_row = consts.tile([1, P], fp32, tag="ones_row")  # bcast lhsT
            nc.vector.memset(ones_row[:], 1.0)
            # all-ones [128,128]: partition-sum WITH broadcast in one matmul
            ones_sq = consts.tile([P, P], fp32, tag="ones_sq")
            nc.vector.memset(ones_sq[:], 1.0)
            for bi in range(1, B_LOC):
                v_sb[bi] = wpool.tile([P, H], fp32, tag=f"v{bi}", name=f"v{bi}")
                nc.scalar.dma_start(v_sb[bi][:], vr[bi])

        # ---- main stream: energies via fused multiply+row-reduce ----
        # l = c*1024 + p*8 + k: each chunk is one contiguous 2 MiB DRAM blob.
        enc_r = enc.rearrange("b (c p k) h -> b c p k h", p=P, k=JCH)

        for bi in range(B_LOC):
            e_sb = epers.tile([P, NCOL], fp32, tag=f"e{bi}")
            for c in range(NCH):
                ci = bi * NCH + c
                et = epool.tile([P, JCH, H], fp32, tag="et")
                ring = nc.scalar if ci % 2 else nc.sync
                if ci == 0:
                    # first chunk: 4 sub-DMAs of 512 KiB so STT k=0 starts
                    # after the first quarter lands instead of the full 2 MiB
                    for j in range(4):
                        r = nc.sync if j % 2 == 0 else nc.scalar
                        r.dma_start(et[:, 2 * j:2 * j + 2, :],
                                    enc_r[bi, c, :, 2 * j:2 * j + 2, :])
                elif ci == 1:
                    for j in range(2):
                        r = nc.scalar if j % 2 == 0 else nc.sync
                        r.dma_start(et[:, 4 * j:4 * j + 4, :],
                                    enc_r[bi, c, :, 4 * j:4 * j + 4, :])
                else:
                    ring.dma_start(et[:], enc_r[bi, c])
                for k in range(JCH):
                    m = c * JCH + k
                    # fused (enc * v) + row-sum in one native op:
                    # out = (in0 * 1.0) * in1 ; accum_out = row_sum(out)
                    if k in POOL_KS:
                        sc = pscratch.tile([P, H], fp32, tag="pttr")
                        eng = nc.gpsimd
                    else:
                        sc = scratch.tile([P, H], fp32, tag="ttr")
                        eng = nc.vector
                    eng.scalar_tensor_tensor(
                        out=sc[:], in0=et[:, k, :], scalar=1.0,
                        in1=v_sb[bi][:],
                        op0=Alu.mult, op1=Alu.mult,
                        accum_out=e_sb[:, m:m + 1],
                    )

            # ---- softmax over the 4096 energies of batch bi ----
            mx = small.tile([P, 1], fp32, tag="mx")
            nc.vector.tensor_reduce(mx[:], e_sb[:], axis=mybir.AxisListType.X,
                                    op=Alu.max)
            mxT_ps = psum.tile([1, P], fp32, tag="red_ps")
            nc.tensor.transpose(mxT_ps[:], mx[:], identity[:])
            ngmax = small.tile([1, 1], fp32, tag="ngmax")
            nc.vector.tensor_reduce(ngmax[:], mxT_ps[:],
                                    axis=mybir.AxisListType.X, op=Alu.max,
                                    negate=True)
            nb_ps = psum.tile([P, 1], fp32, tag="bc_ps")
            nc.tensor.matmul(nb_ps[:], ones_row[:], ngmax[:],
                             start=True, stop=True)
            nbias = small.tile([P, 1], fp32, tag="nbias")
            nc.scalar.copy(nbias[:], nb_ps[:])

            p_sb = epers.tile([P, NCOL], fp32, tag=f"p{bi}")
            ssum = small.tile([P, 1], fp32, tag="ssum")
            nc.scalar.activation(p_sb[:], e_sb[:], Act.Exp,
                                 bias=nbias[:], scale=1.0, accum_out=ssum[:])

            # partition-sum AND broadcast in one matmul: out[m,0] = sum_p ssum
            tot_ps = psum.tile([P, 1], fp32, tag="red_ps")
            nc.tensor.matmul(tot_ps[:], ones_sq[:], ssum[:],
                             start=True, stop=True)
            rbc = small.tile([P, 1], fp32, tag="rbc")
            nc.vector.reciprocal(rbc[:], tot_ps[:])

            o_sb = epers.tile([P, NCOL], fp32, tag=f"o{bi}")
            nc.scalar.mul(o_sb[:], p_sb[:], rbc[:])

            # contiguous 16 KiB store; host inverts the (c,p,k) permutation
            nc.scalar.dma_start(probs[bi], o_sb[:])

    nc.compile()
    return nc


def _get_program():
    global _PROGRAM
    if _PROGRAM is None:
        _PROGRAM = _build_program()
    return _PROGRAM


def _core_inputs(enc, v):
    """Per-core input dicts: enc batch-slice + pre-broadcast v tile."""
    in_maps = []
    for core in range(N_CORES):
        b0 = core * B_LOC
        v_rep = np.ascontiguousarray(
            np.broadcast_to(v[b0:b0 + B_LOC][:, None, :], (B_LOC, P, H)),
            dtype=np.float32)
        in_maps.append({
            "enc": np.ascontiguousarray(enc[b0:b0 + B_LOC]),
            "vr": v_rep,
        })
    return in_maps


def _assemble(probs_list):
    """[B_LOC, P, NCOL] per core -> full [B, 1, L] with l = c*1024 + p*8 + k."""
    full = np.concatenate(probs_list, axis=0)           # [B, P, NCOL]
    out = full.reshape(B, P, NCH, JCH).transpose(0, 2, 1, 3).reshape(B, L)
    return out[:, None, :].astype(np.float32)


def kernel(hidden, encoder_outputs, W, b):
    """Full-input entry point: shards across 8 NeuronCores, returns [B,1,L]."""
    from concourse.bass_utils import run_bass_kernel_spmd

    hidden = np.asarray(hidden, dtype=np.float32)
    enc = np.asarray(encoder_outputs, dtype=np.float32)
    W = np.asarray(W, dtype=np.float32)

    h_last = hidden[0, 1]          # == hidden[0].transpose(1,0,2)[:, -1, :]
    v = (h_last @ W).astype(np.float32)  # [B, H]; bias cancels in softmax

    nc = _get_program()
    in_maps = _core_inputs(enc, v)
    res = run_bass_kernel_spmd(nc, in_maps, list(range(N_CORES)))
    return _assemble([res.results[i]["probs"] for i in range(N_CORES)])
